# revision 11
# baseline (speedup 1.0000x reference)
"""JTNN tree-GRU message passing encoder on 8 trn2 NeuronCores.

Strategy: data-parallel over the per-step message axis E (8192/8 = 1024
rows per core per step). Each core keeps a full replica of the message
state in DRAM as bf16 rows [h[s] | R2[s]] (1KB each) where
R2[s] = h[s] @ Ur + bur is precomputed at message-production time, so the
per-neighbor Ur matmul disappears at consumption time. Per depth step:
indirect-DMA gather of the neighbor rows (an "old rows" pass that can
overlap the previous step's AllGather plus a "fresh rows" pass that
depends on it), GRU compute in bf16 (f32 PSUM accumulate), then an
AllGather of the new rows into every core's replica.

Message rows are stored in "p-major" order within each 512-row tile
(storage_local = p*RC + rc for compute row (p, rc), which holds original
row rc*128 + p) so that every DMA between SBUF [128, RC, *] tiles and
DRAM is contiguous; all neighbor indices are translated to storage order
on the host.
"""

import sys
import numpy as np

sys.path.insert(0, "/opt/trn_rl_repo")
import ml_dtypes

T, E, H, V, B, MAX_NB = 12, 8192, 256, 800, 256, 8
M = T * E
N_CORES = 8
SENTINEL = 0x7FFF0000
BF16 = ml_dtypes.bfloat16


def build_kernel(nc, T_, E_, n_cores, TILE):
    import os
    KSTAGE = int(os.environ.get("KSTAGE", "3"))
    from concourse import bass, mybir
    import concourse.tile as tile_mod
    from concourse.masks import make_identity
    from contextlib import ExitStack

    f32 = mybir.dt.float32
    bf16 = mybir.dt.bfloat16
    i32 = mybir.dt.int32
    AF = mybir.ActivationFunctionType
    ALU = mybir.AluOpType

    E_loc = E_ // n_cores
    n_tiles = E_loc // TILE
    RC = TILE // 128
    M_ = T_ * E_
    B_loc = B // n_cores
    NB = MAX_NB

    # ---------------- DRAM I/O ----------------
    emb = nc.dram_tensor("emb", [V, H], bf16, kind="ExternalInput").ap()
    wz_d = nc.dram_tensor("wz", [128, 4, H], bf16, kind="ExternalInput").ap()
    wr_d = nc.dram_tensor("wr", [128, 2, H], bf16, kind="ExternalInput").ap()
    ur_d = nc.dram_tensor("ur", [128, 2, H], bf16, kind="ExternalInput").ap()
    wh_d = nc.dram_tensor("wh", [128, 4, H], bf16, kind="ExternalInput").ap()
    wo_d = nc.dram_tensor("wo", [128, 4, H], bf16, kind="ExternalInput").ap()
    bz_d = nc.dram_tensor("bz", [128, 2], f32, kind="ExternalInput").ap()
    bh_d = nc.dram_tensor("bh", [128, 2], f32, kind="ExternalInput").ap()
    bo_d = nc.dram_tensor("bo", [128, 2], f32, kind="ExternalInput").ap()
    bur_d = nc.dram_tensor("burb", [128, H], f32, kind="ExternalInput").ap()
    pad_d = nc.dram_tensor("padrow", [1, 2 * H], bf16, kind="ExternalInput").ap()
    gidx_raw_d = nc.dram_tensor(
        "gidx_raw", [128, T_ * n_tiles * NB * RC], i32, kind="ExternalInput"
    ).ap()
    gidx_fr_d = nc.dram_tensor(
        "gidx_fr", [128, T_ * n_tiles * NB * RC], i32, kind="ExternalInput"
    ).ap()
    xidx_d = nc.dram_tensor(
        "xidx", [128, T_ * n_tiles * RC], i32, kind="ExternalInput"
    ).ap()
    ridx_d = nc.dram_tensor("ridx", [128, 2], i32, kind="ExternalInput").ap()
    ridxf_d = nc.dram_tensor("ridxf", [128, 2], i32, kind="ExternalInput").ap()
    rxid_d = nc.dram_tensor("rxid", [128, 1], i32, kind="ExternalInput").ap()
    selw_d = nc.dram_tensor("selw", [128, 2, 32], bf16, kind="ExternalInput").ap()

    h_shard = nc.dram_tensor(
        "h_shard", [T_ * E_loc, H], f32, kind="ExternalOutput"
    ).ap()
    rv_shard = nc.dram_tensor("rv_shard", [128, 2, B_loc], f32, kind="ExternalOutput").ap()

    replica_groups = [list(range(n_cores))]

    with tile_mod.TileContext(nc) as tc:
        ctx = ExitStack()
        with ctx:
            wpool = ctx.enter_context(tc.tile_pool(name="weights", bufs=1))
            gpool = ctx.enter_context(tc.tile_pool(name="gather", bufs=2))
            spool = ctx.enter_context(tc.tile_pool(name="work", bufs=1))
            ppool = ctx.enter_context(tc.tile_pool(name="psum", bufs=1, space="PSUM"))
            dram = ctx.enter_context(tc.tile_pool(name="dram", bufs=1, space="DRAM"))

            # replicated message state [h | R2] rows (tracked pool tile)
            hr = dram.tile([1 + M_, 2 * H], bf16, tag="hr")

            # ---- preload constants ----
            ident = wpool.tile([128, 128], bf16)
            make_identity(nc, ident[:])
            identf = wpool.tile([128, 128], f32)
            make_identity(nc, identf[:])
            wz = wpool.tile([128, 4, H], bf16)
            wr = wpool.tile([128, 2, H], bf16)
            ur = wpool.tile([128, 2, H], bf16)
            wh = wpool.tile([128, 4, H], bf16)
            wo = wpool.tile([128, 4, H], bf16)
            bz = wpool.tile([128, 2], f32)
            bh = wpool.tile([128, 2], f32)
            bo = wpool.tile([128, 2], f32)
            burb = wpool.tile([128, H], f32)
            selw = wpool.tile([128, 2, 32], bf16)
            gidx_raw = wpool.tile([128, T_ * n_tiles * NB * RC], i32)
            gidx_fr = wpool.tile([128, T_ * n_tiles * NB * RC], i32)
            xidx = wpool.tile([128, T_ * n_tiles * RC], i32)
            ridx = wpool.tile([128, 2], i32)
            ridxf = wpool.tile([128, 2], i32)
            rxid = wpool.tile([128, 1], i32)
            for dst, src in [
                (wz, wz_d), (wr, wr_d), (ur, ur_d), (wh, wh_d), (wo, wo_d),
                (bz, bz_d), (bh, bh_d), (bo, bo_d), (burb, bur_d),
                (selw, selw_d), (gidx_raw, gidx_raw_d), (gidx_fr, gidx_fr_d),
                (xidx, xidx_d), (ridx, ridx_d), (ridxf, ridxf_d), (rxid, rxid_d),
            ]:
                nc.sync.dma_start(out=dst[:], in_=src[:])
            nc.sync.dma_start(out=hr[0:1, :], in_=pad_d[:])

            def tr(out_ps, in_sb, fp32=False):
                k = in_sb.shape[0]
                idt = (identf if fp32 else ident)
                nc.tensor.transpose(out=out_ps, in_=in_sb, identity=idt[:k, :k])

            def gather_tile(t, ti, F_prev):
                base_k = t * n_tiles + ti
                g = None
                if t > 0:
                    g = gpool.tile([128, NB * RC, 2 * H], bf16, tag="g")
                    hi = 1 + t * E_
                    for ch in range(NB * RC):
                        col = base_k * NB * RC + ch
                        nc.gpsimd.indirect_dma_start(
                            out=g[:, ch, :],
                            out_offset=None,
                            in_=hr[0:hi, :],
                            in_offset=bass.IndirectOffsetOnAxis(
                                ap=gidx_raw[:, col : col + 1], axis=0
                            ),
                        )
                xg = spool.tile([128, RC, H], bf16, tag="xg", bufs=2)
                for rc in range(RC):
                    col = base_k * RC + rc
                    nc.gpsimd.indirect_dma_start(
                        out=xg[:, rc, :],
                        out_offset=None,
                        in_=emb[:],
                        in_offset=bass.IndirectOffsetOnAxis(
                            ap=xidx[:, col : col + 1], axis=0
                        ),
                    )
                return g, xg

            def step_tile(t, ti, g, xg):
                # ---- x_T ----
                xT_ps = ppool.tile([128, 2, TILE], bf16, tag="tp")
                for rc in range(RC):
                    for fh in range(2):
                        tr(
                            xT_ps[:, fh, rc * 128 : (rc + 1) * 128],
                            xg[:, rc, fh * 128 : (fh + 1) * 128],
                        )
                xT = spool.tile([128, 2, TILE], bf16, tag="xT")
                nc.scalar.activation(xT[:], xT_ps[:], AF.Copy)

                def hsl(nb):
                    return g[:, nb * RC : (nb + 1) * RC, 0:H]

                def r2sl(nb):
                    return g[:, nb * RC : (nb + 1) * RC, H : 2 * H]

                shT = shTf = sgT = None
                if t > 0:
                    # ---- sum_h (serial f32 accumulation, row-major) ----
                    sum_h = spool.tile([128, RC, H], f32, tag="sum_h")
                    nc.vector.tensor_add(sum_h[:], hsl(0), hsl(1))
                    for nb in range(2, NB):
                        nc.vector.tensor_add(sum_h[:], sum_h[:], hsl(nb))
                    shT_ps = ppool.tile([128, 2, TILE], f32, tag="tp")
                    for rc in range(RC):
                        for fh in range(2):
                            tr(
                                shT_ps[:, fh, rc * 128 : (rc + 1) * 128],
                                sum_h[:, rc, fh * 128 : (fh + 1) * 128],
                                fp32=True,
                            )
                    shT = spool.tile([128, 2, TILE], bf16, tag="shT")
                    shTf = spool.tile([128, 2, TILE], f32, tag="shTf")
                    nc.scalar.activation(shT[:], shT_ps[:], AF.Copy)
                    nc.vector.tensor_copy(shTf[:], shT_ps[:])

                # ---- z gate ----
                nk = 4 if t > 0 else 2
                zps = ppool.tile([128, 2, TILE], f32, tag="mm")
                for mh in range(2):
                    for kc in range(nk):
                        rhs = xT[:, kc, :] if kc < 2 else shT[:, kc - 2, :]
                        nc.tensor.matmul(
                            out=zps[:, mh, :],
                            lhsT=wz[:, kc, mh * 128 : (mh + 1) * 128],
                            rhs=rhs,
                            start=(kc == 0),
                            stop=(kc == nk - 1),
                        )
                z = spool.tile([128, 2, TILE], f32, tag="z")
                for mh in range(2):
                    nc.scalar.activation(
                        z[:, mh, :], zps[:, mh, :], AF.Sigmoid, bias=bz[:, mh : mh + 1]
                    )

                if t > 0:
                    # ---- r1 = x @ Wr (row-major) ----
                    r1ps = ppool.tile([128, RC, H], f32, tag="mm2")
                    for rc in range(RC):
                        for kc in range(2):
                            nc.tensor.matmul(
                                out=r1ps[:, rc, :],
                                lhsT=xT[:, kc, rc * 128 : (rc + 1) * 128],
                                rhs=wr[:, kc, :],
                                start=(kc == 0),
                                stop=(kc == 1),
                            )
                    r1 = spool.tile([128, RC, H], bf16, tag="r1")
                    nc.scalar.activation(r1[:], r1ps[:], AF.Copy)

                    # ---- per-neighbor r = sigmoid(r1 + R2g); sum r*h ----
                    sumg = spool.tile([128, RC, H], f32, tag="sumg")
                    prev = None
                    for nb in range(NB):
                        rp = spool.tile([128, RC, H], bf16, tag=f"rp{nb % 2}",
                                        bufs=2)
                        nc.gpsimd.tensor_tensor(
                            out=rp[:], in0=r1[:], in1=r2sl(nb), op=ALU.add
                        )
                        rr = spool.tile([128, RC, H], bf16, tag=f"rr{nb % 2}",
                                        bufs=2)
                        nc.scalar.activation(rr[:], rp[:], AF.Sigmoid)
                        pr = spool.tile([128, RC, H], bf16, tag=f"pr{nb % 2}",
                                        bufs=2)
                        nc.vector.tensor_mul(pr[:], rr[:], hsl(nb))
                        if nb == 1:
                            nc.vector.tensor_add(sumg[:], prev[:], pr[:])
                        elif nb > 1:
                            nc.vector.tensor_add(sumg[:], sumg[:], pr[:])
                        prev = pr
                    sgT_ps = ppool.tile([128, 2, TILE], f32, tag="tp")
                    for rc in range(RC):
                        for fh in range(2):
                            tr(
                                sgT_ps[:, fh, rc * 128 : (rc + 1) * 128],
                                sumg[:, rc, fh * 128 : (fh + 1) * 128],
                                fp32=True,
                            )
                    sgT = spool.tile([128, 2, TILE], bf16, tag="sgT")
                    nc.scalar.activation(sgT[:], sgT_ps[:], AF.Copy)

                # ---- pre_h ----
                php = ppool.tile([128, 2, TILE], f32, tag="mm")
                for mh in range(2):
                    for kc in range(nk):
                        rhs = xT[:, kc, :] if kc < 2 else sgT[:, kc - 2, :]
                        nc.tensor.matmul(
                            out=php[:, mh, :],
                            lhsT=wh[:, kc, mh * 128 : (mh + 1) * 128],
                            rhs=rhs,
                            start=(kc == 0),
                            stop=(kc == nk - 1),
                        )
                ph = spool.tile([128, 2, TILE], f32, tag="ph")
                for mh in range(2):
                    nc.scalar.activation(
                        ph[:, mh, :], php[:, mh, :], AF.Tanh, bias=bh[:, mh : mh + 1]
                    )

                # ---- combine: new_h = sum_h + z*(ph - sum_h); z*ph at t=0 ----
                nhT = spool.tile([128, 2, TILE], bf16, tag="nhT")
                if t > 0:
                    d = spool.tile([128, 2, TILE], f32, tag="d")
                    nc.vector.tensor_sub(d[:], ph[:], shTf[:])
                    nc.vector.tensor_mul(d[:], z[:], d[:])
                    nc.vector.tensor_add(nhT[:], shTf[:], d[:])
                else:
                    nc.vector.tensor_mul(nhT[:], z[:], ph[:])

                # ---- detranspose new_h -> row-major (p-major rows) ----
                nh_ps = ppool.tile([128, RC, H], bf16, tag="nh")
                for mh in range(2):
                    for rc in range(RC):
                        tr(
                            nh_ps[:, rc, mh * 128 : (mh + 1) * 128],
                            nhT[:, mh, rc * 128 : (rc + 1) * 128],
                        )
                nh_row = spool.tile([128, RC, H], f32, tag="nh_row", bufs=2)
                nc.scalar.activation(nh_row[:], nh_ps[:], AF.Copy)
                row0 = t * E_loc + ti * TILE
                nc.sync.dma_start(
                    out=h_shard[row0 : row0 + TILE, :], in_=nh_row[:]
                )
                hrblk = spool.tile([128, RC, 2 * H], bf16, tag="hrblk", bufs=2)
                nc.vector.tensor_copy(hrblk[:, :, 0:H], nh_row[:])
                # R2 = new_h @ Ur + bur
                r2ps = ppool.tile([128, RC, H], f32, tag="mm2")
                for rc in range(RC):
                    for kc in range(2):
                        nc.tensor.matmul(
                            out=r2ps[:, rc, :],
                            lhsT=nhT[:, kc, rc * 128 : (rc + 1) * 128],
                            rhs=ur[:, kc, :],
                            start=(kc == 0),
                            stop=(kc == 1),
                        )
                for rc in range(RC):
                    nc.vector.tensor_tensor(
                        out=hrblk[:, rc, H : 2 * H],
                        in0=r2ps[:, rc, :],
                        in1=burb[:],
                        op=ALU.add,
                    )
                return hrblk

            Fs = []
            for t in range(T_):
                F_prev = Fs[t - 1] if t > 0 else None
                gx = [gather_tile(t, ti, F_prev) for ti in range(n_tiles)]
                cin = dram.tile([E_loc, 2 * H], bf16, tag=f"cin{t}")
                for ti in range(n_tiles):
                    hrblk = step_tile(t, ti, *gx[ti])
                    nc.sync.dma_start(
                        out=cin[ti * TILE : (ti + 1) * TILE, :], in_=hrblk[:]
                    )
                F_t = dram.tile([E_, 2 * H], bf16, tag=f"F{t}",
                                addr_space="Shared")
                Fs.append(F_t)
                if KSTAGE < 3:
                    continue
                nc.gpsimd.collective_compute(
                    "AllGather",
                    mybir.AluOpType.bypass,
                    ins=[cin[:].opt()],
                    outs=[F_t[:].opt()],
                    replica_groups=replica_groups,
                )
                nc.sync.dma_start(
                    out=hr[1 + t * E_ : 1 + (t + 1) * E_, :], in_=F_t[:]
                )

            # ---------------- root aggregation ----------------
            if KSTAGE < 4:
                rv0 = spool.tile([128, 2, B_loc], f32, tag="rv0")
                nc.vector.memset(rv0[:], 0)
                nc.sync.dma_start(out=rv_shard[:], in_=rv0[:])
                return
            gr = spool.tile([128, 2, 2 * H], bf16, tag="gr")
            for ch in range(2):
                nc.gpsimd.indirect_dma_start(
                    out=gr[:, ch, :],
                    out_offset=None,
                    in_=hr[:],
                    in_offset=bass.IndirectOffsetOnAxis(
                        ap=ridx[:, ch : ch + 1], axis=0
                    ),
                )
            xr = spool.tile([128, 1, H], bf16, tag="xr")
            nc.gpsimd.indirect_dma_start(
                out=xr[:],
                out_offset=None,
                in_=emb[:],
                in_offset=bass.IndirectOffsetOnAxis(ap=rxid[:], axis=0),
            )
            # snT[f, b] = sum over each root's 8 gathered rows (via selector)
            snT_ps = ppool.tile([128, 2, 32], f32, tag="mm2")
            for mh in range(2):
                for ch in range(2):
                    nc.tensor.matmul(
                        out=snT_ps[:, mh, :],
                        lhsT=gr[:, ch, mh * 128 : (mh + 1) * 128],
                        rhs=selw[:, ch, :],
                        start=(ch == 0),
                        stop=(ch == 1),
                    )
            snT = spool.tile([128, 2, 32], bf16, tag="snT")
            nc.scalar.activation(snT[:], snT_ps[:], AF.Copy)
            # xrT[f, b] = x_root transposed via identity-select matmul
            xrT_ps = ppool.tile([128, 2, 32], f32, tag="mm")
            for mh in range(2):
                nc.tensor.matmul(
                    out=xrT_ps[:, mh, :],
                    lhsT=xr[:, 0, mh * 128 : (mh + 1) * 128],
                    rhs=ident[:, 0:32],
                    start=True,
                    stop=True,
                )
            xrT = spool.tile([128, 2, 32], bf16, tag="xrT")
            nc.scalar.activation(xrT[:], xrT_ps[:], AF.Copy)
            rvps = ppool.tile([128, 2, 32], f32, tag="nh")
            for mh in range(2):
                for kc in range(4):
                    rhs = xrT[:, kc, :] if kc < 2 else snT[:, kc - 2, :]
                    nc.tensor.matmul(
                        out=rvps[:, mh, :],
                        lhsT=wo[:, kc, mh * 128 : (mh + 1) * 128],
                        rhs=rhs,
                        start=(kc == 0),
                        stop=(kc == 3),
                    )
            rvT = spool.tile([128, 2, 32], f32, tag="rvT")
            for mh in range(2):
                nc.scalar.activation(
                    rvT[:, mh, :], rvps[:, mh, :], AF.Relu, bias=bo[:, mh : mh + 1]
                )
            nc.sync.dma_start(out=rv_shard[:], in_=rvT[:])


# ---------------------------------------------------------------------------
# Host side
# ---------------------------------------------------------------------------

def _storage_perm(E_loc, TILE):
    """orig local row index for each storage row: stor = ti*TILE + p*RC + rc
    holds orig = ti*TILE + rc*128 + p."""
    RC = TILE // 128
    n_tiles = E_loc // TILE
    p = np.arange(128)
    rc = np.arange(RC)
    orig_of_stor = np.empty(E_loc, np.int64)
    for ti in range(n_tiles):
        blk = (rc[None, :] * 128 + p[:, None]).reshape(-1)  # [p, rc] -> orig in tile
        orig_of_stor[ti * TILE : (ti + 1) * TILE] = ti * TILE + blk
    return orig_of_stor


def _translate(s_ref, T_, E_, n_cores, TILE):
    """Map reference slot ids -> storage slot ids (vectorized)."""
    E_loc = E_ // n_cores
    RC = TILE // 128
    s = np.asarray(s_ref, np.int64)
    r = s - 1
    e = r % E_
    el = e % E_loc
    r3 = el % TILE
    sto_local = (r3 % 128) * RC + r3 // 128
    out = 1 + (r - el) + (el - r3) + sto_local
    return np.where(s == 0, 0, out).astype(np.int32)


def _host_prep(inputs, T_, E_, n_cores, TILE):
    E_loc = E_ // n_cores
    n_tiles = E_loc // TILE
    RC = TILE // 128
    B_loc = B // n_cores
    NB = MAX_NB

    Wz, Wr, Ur, Wh, Wo = (np.asarray(inputs[k], np.float32)
                          for k in ("Wz", "Wr", "Ur", "Wh", "Wo"))
    bz, bh, bo, bur = (np.asarray(inputs[k], np.float32)
                       for k in ("bz", "bh", "bo", "bur"))
    embedding = np.asarray(inputs["embedding"], np.float32)

    def lhsT_chunks(W, kparts):
        return np.ascontiguousarray(
            W.reshape(kparts, 128, H).transpose(1, 0, 2)
        ).astype(BF16)

    def bias_T(b):
        return np.ascontiguousarray(b.reshape(2, 128).T, dtype=np.float32)

    wz_h, wr_h, ur_h = lhsT_chunks(Wz, 4), lhsT_chunks(Wr, 2), lhsT_chunks(Ur, 2)
    wh_h, wo_h = lhsT_chunks(Wh, 4), lhsT_chunks(Wo, 4)
    emb_h = embedding.astype(BF16)
    bz_h, bh_h, bo_h = bias_T(bz), bias_T(bh), bias_T(bo)
    bur_b = np.ascontiguousarray(np.broadcast_to(bur[None, :], (128, H)),
                                 dtype=np.float32)
    pad = np.zeros((1, 2 * H), BF16)
    pad[0, H:] = bur.astype(BF16)
    selw_h = np.zeros((128, 2, 32), np.float32)
    for ch in range(2):
        selw_h[np.arange(128), ch, ch * 16 + np.arange(128) // 8] = 1.0
    selw_h = selw_h.astype(BF16)

    x_ids = np.asarray(inputs["x_ids"], np.int64)
    nei_ref = np.asarray(inputs["nei_idx"], np.int64)
    root_wid = np.asarray(inputs["root_wid"], np.int64)
    root_nei = np.asarray(inputs["root_nei"], np.int64)
    nei_sto = _translate(nei_ref, T_, E_, n_cores, TILE).astype(np.int32)
    rn_sto = _translate(root_nei, T_, E_, n_cores, TILE).astype(np.int32)

    in_maps = []
    for c in range(n_cores):
        xs = x_ids[:, c * E_loc : (c + 1) * E_loc].astype(np.int32)
        ns = nei_sto[:, c * E_loc : (c + 1) * E_loc, :]
        ns_ref = nei_ref[:, c * E_loc : (c + 1) * E_loc, :]
        # [T, n_tiles, RC, 128, NB] -> gather layout [128, T*n_tiles*NB*RC]
        nr = ns.reshape(T_, n_tiles, RC, 128, NB)
        nrr = ns_ref.reshape(T_, n_tiles, RC, 128, NB)
        graw = nr.transpose(3, 0, 1, 4, 2)  # [128, T, n_tiles, NB, RC]
        grefr = nrr.transpose(3, 0, 1, 4, 2)
        thr = (np.arange(T_, dtype=np.int64) - 1) * E_
        fresh = grefr > thr[None, :, None, None, None]
        rel = graw - (1 + thr[None, :, None, None, None]).astype(np.int64)
        gfr = np.where(fresh, rel, np.int64(SENTINEL))
        graw_h = np.ascontiguousarray(graw.reshape(128, -1), dtype=np.int32)
        gfr_h = np.ascontiguousarray(gfr.reshape(128, -1), dtype=np.int32)
        xr_ = xs.reshape(T_, n_tiles, RC, 128)
        xh = np.ascontiguousarray(
            xr_.transpose(3, 0, 1, 2).reshape(128, -1), dtype=np.int32
        )
        rn = rn_sto[c * B_loc : (c + 1) * B_loc, :].reshape(-1)
        rn_ref_c = root_nei[c * B_loc : (c + 1) * B_loc, :].reshape(-1)
        rn_f = np.where(
            rn_ref_c > (T_ - 1) * E_, rn - (1 + (T_ - 1) * E_),
            np.int64(SENTINEL),
        ).astype(np.int32)
        ridx_h = np.zeros((128, 2), np.int32)
        ridxf_h = np.full((128, 2), SENTINEL, np.int32)
        ridx_h[: min(128, rn.size), 0] = rn[:128]
        ridxf_h[: min(128, rn.size), 0] = rn_f[:128]
        if rn.size > 128:
            ridx_h[: rn.size - 128, 1] = rn[128:]
            ridxf_h[: rn.size - 128, 1] = rn_f[128:]
        rxid_h = np.zeros((128, 1), np.int32)
        rxid_h[:B_loc, 0] = root_wid[c * B_loc : (c + 1) * B_loc].astype(np.int32)
        in_maps.append({
            "emb": emb_h, "wz": wz_h, "wr": wr_h, "ur": ur_h, "wh": wh_h,
            "wo": wo_h, "bz": bz_h, "bh": bh_h, "bo": bo_h, "burb": bur_b,
            "padrow": pad, "gidx_raw": graw_h, "gidx_fr": gfr_h, "xidx": xh,
            "ridx": ridx_h, "ridxf": ridxf_h, "rxid": rxid_h, "selw": selw_h,
        })
    return in_maps


_CACHE = {}


def run_cores(inputs, T_=T, E_=E, n_cores=N_CORES, TILE=512, trace=False):
    from concourse import bacc, bass_utils

    key = (T_, E_, n_cores, TILE)
    if key not in _CACHE:
        nc = bacc.Bacc(
            "TRN2", target_bir_lowering=False, debug=False, num_devices=n_cores
        )
        build_kernel(nc, T_, E_, n_cores, TILE)
        nc.compile()
        _CACHE[key] = nc
    nc = _CACHE[key]
    in_maps = _host_prep(inputs, T_, E_, n_cores, TILE)
    res = bass_utils.run_bass_kernel_spmd(
        nc, in_maps, list(range(n_cores)), trace=trace
    )
    return res


def assemble(results, T_=T, E_=E, n_cores=N_CORES, TILE=512):
    E_loc = E_ // n_cores
    B_loc = B // n_cores
    M_ = T_ * E_
    orig_of_stor = _storage_perm(E_loc, TILE)
    h = np.zeros((1 + M_, H), np.float32)
    rv = np.zeros((B, H), np.float32)
    for c in range(n_cores):
        shard = results[c]["h_shard"].reshape(T_, E_loc, H)
        blk = np.empty((E_loc, H), np.float32)
        for t in range(T_):
            blk[orig_of_stor] = shard[t]
            h[1 + t * E_ + c * E_loc : 1 + t * E_ + (c + 1) * E_loc] = blk
        v = results[c]["rv_shard"]  # [128, 2, B_loc]
        rv[c * B_loc : (c + 1) * B_loc] = (
            v.transpose(2, 1, 0).reshape(B_loc, H)
        )
    return h, rv


def _host_root(h, inputs):
    emb = np.asarray(inputs["embedding"], np.float32)
    Wo = np.asarray(inputs["Wo"], np.float32)
    bo = np.asarray(inputs["bo"], np.float32)
    x_root = emb[np.asarray(inputs["root_wid"])]
    sum_nei = h[np.asarray(inputs["root_nei"])].sum(1)
    a = np.concatenate([x_root, sum_nei], -1) @ Wo + bo
    return np.maximum(a, 0.0)


def kernel(**inputs):
    inputs = {k: np.asarray(v) for k, v in inputs.items()}
    res = run_cores(inputs)
    h, _ = assemble(res.results)
    rv = _host_root(h, inputs)
    return h, rv
